# revision 3
# baseline (speedup 1.0000x reference)
"""Distributed Trainium2 (Bass/Tile) kernel for masked GAT-style attention.

Reference computation (H=4 heads, N=4096 nodes, D=128):
    scores = leaky_relu(x @ W^T + b, 0.2)            # [H, N, N]
    att    = where(mask, softmax(where(mask, scores, -inf)), 0)
    out    = att @ x                                  # [H, N, D]

Sharding: 8 cores = 4 heads x 2 row-blocks of 2048 nodes. Each core
computes out[h, r0:r0+2048] independently (no collectives).

Per-core layout ("transposed scores"): scores^T tiles [m=128 part, n free]
so the PV matmul uses the attention tile directly as the stationary
operand and the softmax row-sum comes for free from an appended
ones-column on x.

The 32 m-tiles per chunk are split into three per-tile pipelines chosen
to balance ACT (1.2 GHz, 1x) and DVE (0.96 GHz, STT=1x, TT=2x @16-bit):
- A (12 tiles, in quads): ACT Prelu from PSUM -> one shared 4-tile Exp
  on ACT -> fp16 mask multiply on DVE (2x TT). Exact exp.
- E (13 tiles): ACT Prelu(scale=K) gives K*leaky(s) in fp16; one DVE
  TT-add with a host-baked fp16 mask tensor {B, B-30K} produces
  Schraudolph exp bits in uint16 (negative results saturate to 0 =
  masked-out entries vanish); bitcast fp16 feeds the PV directly.
- C (7 tiles): all-DVE. STT t=0.2K*s+mask (PSUM read), STT u=max(5t,t),
  then a 4x-mode tensor_scalar add-B -> uint16 Schraudolph bits.
Schraudolph tiles (E+C) trade ~2% per-entry exp noise (which largely
cancels in the softmax) for removing the ACT exp pass; measured
end-to-end rel err ~1.2e-2 vs the 2e-2 gate.

Mask ships once as uint16 [N, ROWS]: fp16 1.0/0.0 rows for A tiles,
fp16 {B, B-30K} for E, fp16 {0, -44320} for C; bitcast on chip.
Mask/xa DMAs are batched and issued from the otherwise-idle GpSimd
sequencer. wt/xt are split into 512-col pieces so the first QK matmul
starts ~1.5us after launch.
"""

import sys

if "/opt/trn_rl_repo" not in sys.path:
    sys.path.insert(0, "/opt/trn_rl_repo")

import numpy as np
import ml_dtypes

import concourse.bass as bass
import concourse.tile as tile
from concourse import bacc, mybir
from concourse.bass_utils import run_bass_kernel_spmd

BF = mybir.dt.bfloat16
F16 = mybir.dt.float16
F32 = mybir.dt.float32
U16 = mybir.dt.uint16
BF_NP = ml_dtypes.bfloat16

H, N, D = 4, 4096, 128
N_CORES = 8
ROWS = N * H // N_CORES          # 2048 rows (n) per core
CHUNK = 1024                     # n columns processed per outer chunk
CHUNKS = ROWS // CHUNK           # 2
M_TILES = N // 128               # 32 tiles along the softmax (m) axis
SUBS = CHUNK // 128              # 8 PV subtiles per chunk
MB = 4                           # m-tiles per batched mask DMA

# Schraudolph constants (fp16 target: 10 mantissa bits)
SK = 1024.0 / float(np.log(2.0))     # 1477.32
BE = 15296.0                          # exp bias for E tiles (fp16-exact)
BC = 15299.0                          # exp bias for C tiles
E_MASKED = -29024.0                   # fp16(BE - 30*SK)
C_MASKED = -44320.0                   # fp16(-30*SK)

# Module-level knobs used by test.py; harmless defaults for grading.
TRACE = False
LAST_EXEC_NS = None

_CACHED_NC = None
_CACHED_BIAS = None


def _classes(has_bias):
    """Per-m-tile pipeline class: 'A' exact, 'E'/'C' Schraudolph."""
    if has_bias:
        return ["A"] * M_TILES
    cls = ["E"] * M_TILES
    for m in (list(range(0, 4)) + list(range(12, 16)) + list(range(24, 28))):
        cls[m] = "A"
    for m in (5, 8, 11, 17, 20, 23, 29):
        cls[m] = "C"
    return cls


def _build_nc(has_bias=False):
    nc = bacc.Bacc("TRN2", target_bir_lowering=False, debug=False,
                   num_devices=N_CORES)
    xt_d = nc.dram_tensor("xt", [128, ROWS], BF, kind="ExternalInput").ap()
    wt_d = nc.dram_tensor("wt", [128, N], BF, kind="ExternalInput").ap()
    xa_d = nc.dram_tensor("xa", [N, D + 1], F16, kind="ExternalInput").ap()
    mk_d = nc.dram_tensor("mk", [N, ROWS], U16, kind="ExternalInput").ap()
    bc_d = nc.dram_tensor("bc", [128, M_TILES], F32, kind="ExternalInput").ap()
    out_d = nc.dram_tensor("out", [ROWS, D], F32, kind="ExternalOutput").ap()

    PRELU = mybir.ActivationFunctionType.Prelu
    EXP = mybir.ActivationFunctionType.Exp
    COPY = mybir.ActivationFunctionType.Copy
    MUL = mybir.AluOpType.mult
    ADD = mybir.AluOpType.add
    MAX = mybir.AluOpType.max

    CLS = _classes(has_bias)
    # consecutive A runs for shared exp instructions
    a_groups = []
    run = []
    for m in range(M_TILES):
        if CLS[m] == "A":
            run.append(m)
        elif run:
            a_groups.append(run)
            run = []
    if run:
        a_groups.append(run)
    group_of = {}
    for g in a_groups:
        for m in g:
            group_of[m] = g

    WPC = 512                        # wt DMA piece columns (4 m-tiles)
    n_wt = N // WPC                  # 8 stationary pieces
    XG = 8                           # m-tiles per xa group
    n_xa = M_TILES // XG             # 4 PV-moving groups

    with tile.TileContext(nc) as tc:
        with (
            tc.tile_pool(name="const", bufs=1) as cpool,
            tc.tile_pool(name="mask", bufs=7) as mpool,
            tc.tile_pool(name="worka", bufs=2) as apool,
            tc.tile_pool(name="worke", bufs=5) as epool,
            tc.tile_pool(name="workc", bufs=3) as qpool,
            tc.tile_pool(name="outp", bufs=3) as opool,
            tc.tile_pool(name="spsum", bufs=2, space="PSUM") as spool,
            tc.tile_pool(name="opsum", bufs=1, space="PSUM") as oppool,
        ):
            wt_sb = [cpool.tile([128, WPC], BF, name=f"wt{i}")
                     for i in range(n_wt)]
            xt_sb = [cpool.tile([128, 512], BF, name=f"xt{i}")
                     for i in range(2 * CHUNKS)]
            bc_sb = cpool.tile([128, M_TILES], F32)
            xa_sb = [cpool.tile([128, XG, D + 1], F16, name=f"xa{i}")
                     for i in range(n_xa)]

            # Warm the ACT LUT set while input DMAs are in flight.
            warm_sb = cpool.tile([1, 1], F32)
            nc.scalar.activation(warm_sb[:], warm_sb[:], EXP)

            # First pieces needed by the first QK matmul go first.
            nc.sync.dma_start(out=wt_sb[0][:], in_=wt_d[:, 0:WPC])
            nc.sync.dma_start(out=xt_sb[0][:], in_=xt_d[:, 0:512])
            if has_bias:
                nc.sync.dma_start(out=bc_sb[:], in_=bc_d[:, :])
            xa_v = xa_d.rearrange("(g j p) d -> g p j d", p=128, j=XG)
            for i in range(1, 2 * CHUNKS):
                nc.sync.dma_start(out=xt_sb[i][:],
                                  in_=xt_d[:, i * 512:(i + 1) * 512])
            for i in range(1, n_wt):
                nc.sync.dma_start(out=wt_sb[i][:],
                                  in_=wt_d[:, i * WPC:(i + 1) * WPC])

            pending_tail = None
            for c in range(CHUNKS):
                o_ps = []

                def o_ap(s, o_ps=o_ps):
                    return o_ps[s // 3][:, (s % 3) * 129:(s % 3) * 129 + 129]

                def emit_pv(m, lhs_t, first=False):
                    if first:
                        if pending_tail is not None:
                            pending_tail()
                        o_ps.extend(
                            oppool.tile([128, 512], F32, tag=f"oacc{b}",
                                        name=f"oacc{b}_c{c}")
                            for b in range((SUBS + 2) // 3))
                    for s in range(SUBS):
                        nc.tensor.matmul(
                            o_ap(s),
                            lhsT=lhs_t[:, s * 128:(s + 1) * 128],
                            rhs=xa_sb[m // XG][:, m % XG],
                            start=(m == 0 and s % 3 == 0),
                            stop=(m == M_TILES - 1),
                            skip_group_check=True,
                        )

                mkb_sb = None
                lq_sb = None
                first_pv = True
                for m in range(M_TILES):
                    # Batched mask DMA, alternating Sync / GpSimd issue.
                    if m % MB == 0:
                        mkb_sb = mpool.tile([128, MB, CHUNK], U16,
                                            tag="mkb", name=f"mkb_c{c}_m{m}")
                        mk_v = mk_d[m * 128:(m + MB) * 128,
                                    c * CHUNK:(c + 1) * CHUNK].rearrange(
                                        "(t p) n -> p t n", p=128)
                        eng = nc.gpsimd if (m // MB) % 2 == 0 else nc.sync
                        eng.dma_start(out=mkb_sb[:], in_=mk_v)
                        if c == 0 and m == 0:
                            nc.gpsimd.dma_start(out=xa_sb[0][:], in_=xa_v[0])
                        if c == 0 and m == MB:
                            for g in range(1, n_xa):
                                nc.gpsimd.dma_start(out=xa_sb[g][:],
                                                    in_=xa_v[g])
                    mk_sb = mkb_sb[:, m % MB]

                    # QK: scores^T tile [m=128, n=CHUNK] in PSUM.
                    s_ps = spool.tile([128, CHUNK], F32, tag="s",
                                      name=f"s_c{c}_m{m}")
                    for half in range(CHUNK // 512):
                        nc.tensor.matmul(
                            s_ps[:, half * 512:(half + 1) * 512],
                            lhsT=wt_sb[(m * 128) // WPC]
                                 [:, (m * 128) % WPC:(m * 128) % WPC + 128],
                            rhs=xt_sb[c * 2 + half][:],
                            start=True, stop=True,
                        )

                    cls = CLS[m]
                    if cls == "A":
                        grp = group_of[m]
                        gi = grp.index(m)
                        if gi == 0:
                            lq_sb = apool.tile([128, len(grp), CHUNK], F16,
                                               tag="lq", name=f"lq_c{c}_m{m}")
                        bias = bc_sb[:, m:m + 1] if has_bias else 0.0
                        nc.scalar.activation(lq_sb[:, gi], s_ps[:], PRELU,
                                             bias=bias, scale=1.0, alpha=0.2)
                        if gi == len(grp) - 1:
                            eq_sb = apool.tile([128, len(grp), CHUNK], F16,
                                               tag="eq", name=f"eq_c{c}_m{m}")
                            nc.scalar.activation(eq_sb[:], lq_sb[:], EXP)
                            for gj, mj in enumerate(grp):
                                mkj = mkb_sb[:, mj % MB]
                                a_sb = epool.tile([128, CHUNK], F16, tag="a",
                                                  name=f"a_c{c}_m{mj}")
                                nc.vector.tensor_mul(a_sb[:], eq_sb[:, gj],
                                                     mkj.bitcast(F16))
                                emit_pv(mj, a_sb[:], first=first_pv)
                                if first_pv:
                                    first_pv = False
                                    pending_tail = None
                    elif cls == "E":
                        lp_sb = epool.tile([128, CHUNK], F16, tag="lp",
                                           name=f"lp_c{c}_m{m}")
                        nc.scalar.activation(lp_sb[:], s_ps[:], PRELU,
                                             bias=0.0, scale=SK, alpha=0.2)
                        eb_sb = epool.tile([128, CHUNK], U16, tag="eb",
                                           name=f"eb_c{c}_m{m}")
                        nc.vector.tensor_tensor(eb_sb[:], lp_sb[:],
                                                mk_sb.bitcast(F16), ADD)
                        emit_pv(m, eb_sb[:].bitcast(F16), first=first_pv)
                        first_pv = False
                    else:  # C
                        t_sb = qpool.tile([128, CHUNK], F16, tag="ct",
                                          name=f"t_c{c}_m{m}")
                        nc.vector.scalar_tensor_tensor(
                            t_sb[:], s_ps[:], 0.2 * SK, mk_sb.bitcast(F16),
                            op0=MUL, op1=ADD)
                        u_sb = qpool.tile([128, CHUNK], F16, tag="cu",
                                          name=f"u_c{c}_m{m}")
                        nc.vector.scalar_tensor_tensor(
                            u_sb[:], t_sb[:], 5.0, t_sb[:],
                            op0=MUL, op1=MAX)
                        cb_sb = qpool.tile([128, CHUNK], U16, tag="cb",
                                           name=f"cb_c{c}_m{m}")
                        nc.vector.tensor_scalar(cb_sb[:], u_sb[:], BC, 0.0,
                                                op0=ADD, op1=MAX)
                        emit_pv(m, cb_sb[:].bitcast(F16), first=first_pv)
                        first_pv = False

                def make_tail(c=c, o_ap=o_ap):
                    def emit_tail():
                        # Division tail: reciprocal of the free row-sum,
                        # scale-copy gathered into one tile per chunk.
                        of_big = opool.tile([128, SUBS, D], F32, tag="ofbig",
                                            name=f"ofbig_c{c}")
                        r_sbs = []
                        for s in range(SUBS):
                            r_sb = opool.tile([128, 1], F32, tag=f"recip{s}",
                                              name=f"recip_c{c}_s{s}")
                            nc.vector.reciprocal(r_sb[:], o_ap(s)[:, 128:129])
                            r_sbs.append(r_sb)
                        for s in range(SUBS):
                            on_act_scale = (c == CHUNKS - 1) and s % 2 == 0
                            if on_act_scale:
                                nc.scalar.activation(of_big[:, s],
                                                     o_ap(s)[:, 0:D],
                                                     COPY, bias=0.0,
                                                     scale=r_sbs[s][:])
                            else:
                                nc.vector.tensor_scalar_mul(of_big[:, s],
                                                            o_ap(s)[:, 0:D],
                                                            r_sbs[s][:])
                        halves = 2 if c == CHUNKS - 1 else 1
                        hs = SUBS // halves
                        for hh in range(halves):
                            row0 = c * CHUNK + hh * hs * 128
                            out_v = out_d[row0:row0 + hs * 128, :].rearrange(
                                "(s p) d -> p s d", p=128)
                            eng = nc.scalar if hh == 1 else nc.sync
                            eng.dma_start(out=out_v,
                                          in_=of_big[:, hh * hs:(hh + 1) * hs])
                    return emit_tail
                pending_tail = make_tail()
            if pending_tail is not None:
                pending_tail()

    nc.compile()
    return nc


def _pack_mask(mask_t_u8, has_bias):
    """mask_t_u8: [N, ROWS] 0/1. Returns uint16-packed per-m-tile rows."""
    cls = _classes(has_bias)
    out = np.empty(mask_t_u8.shape, np.uint16)
    for m in range(M_TILES):
        rows = slice(m * 128, (m + 1) * 128)
        blk = mask_t_u8[rows].astype(np.float32)
        if cls[m] == "A":
            vals = blk
        elif cls[m] == "E":
            vals = np.where(blk > 0, BE, E_MASKED)
        else:
            vals = np.where(blk > 0, 0.0, C_MASKED)
        out[rows] = vals.astype(np.float16).view(np.uint16)
    return out


def kernel(x, W, b, neighbor_mask):
    global _CACHED_NC, _CACHED_BIAS, LAST_EXEC_NS
    x = np.asarray(x, dtype=np.float32)
    W = np.asarray(W, dtype=np.float32)
    b = np.asarray(b, dtype=np.float32)
    mask = np.asarray(neighbor_mask)

    has_bias = bool(np.any(b))
    if _CACHED_NC is None or _CACHED_BIAS != has_bias:
        _CACHED_NC = _build_nc(has_bias=has_bias)
        _CACHED_BIAS = has_bias
    nc = _CACHED_NC

    mask_u8 = mask.astype(np.uint8)
    in_maps = []
    for core in range(N_CORES):
        h, rb = core // 2, core % 2
        r0 = rb * ROWS
        xt = np.ascontiguousarray(x[h, r0:r0 + ROWS].T).astype(BF_NP)
        wt = np.ascontiguousarray(W[h].T).astype(BF_NP)
        xa = np.concatenate(
            [x[h], np.ones((N, 1), np.float32)], axis=1
        ).astype(np.float16)
        mk = _pack_mask(
            np.ascontiguousarray(mask_u8[r0:r0 + ROWS].T), has_bias)
        bc = np.ascontiguousarray(b[h].reshape(M_TILES, 128).T)
        in_maps.append({"xt": xt, "wt": wt, "xa": xa, "mk": mk, "bc": bc})

    res = run_bass_kernel_spmd(nc, in_maps, core_ids=list(range(N_CORES)),
                               trace=TRACE)
    LAST_EXEC_NS = res.exec_time_ns

    out = np.empty((H, N, D), np.float32)
    for core in range(N_CORES):
        h, rb = core // 2, core % 2
        r0 = rb * ROWS
        out[h, r0:r0 + ROWS] = res.results[core]["out"]
    return out


# revision 11
# speedup vs baseline: 1.2100x; 1.2100x over previous
"""Distributed Trainium2 (Bass/Tile) kernel for masked GAT-style attention.

Reference computation (H=4 heads, N=4096 nodes, D=128):
    scores = leaky_relu(x @ W^T + b, 0.2)            # [H, N, N]
    att    = where(mask, softmax(where(mask, scores, -inf)), 0)
    out    = att @ x                                  # [H, N, D]

Sharding: 8 cores = 4 heads x 2 row-blocks of 2048 nodes. Each core
computes out[h, r0:r0+2048] independently (no collectives).

Per-core layout ("transposed scores"): scores^T tiles [m=128 part, n free]
so the PV matmul uses the attention tile directly as the stationary
operand and the softmax row-sum comes for free from an appended
ones-column on x.

The 32 m-tiles per chunk are split into three per-tile pipelines chosen
to balance ACT (1.2 GHz, 1x) and DVE (0.96 GHz, STT=1x, TT=2x @16-bit):
- A (12 tiles, in quads): ACT Prelu from PSUM -> one shared 4-tile Exp
  on ACT -> fp16 mask multiply on DVE (2x TT). Exact exp.
- E (13 tiles): ACT Prelu(scale=K) gives K*leaky(s) in fp16; one DVE
  TT-add with a host-baked fp16 mask tensor {B, B-30K} produces
  Schraudolph exp bits in uint16 (negative results saturate to 0 =
  masked-out entries vanish); bitcast fp16 feeds the PV directly.
- C (7 tiles): all-DVE. STT t=0.2K*s+mask (PSUM read), STT u=max(5t,t),
  then a 4x-mode tensor_scalar add-B -> uint16 Schraudolph bits.
Schraudolph tiles (E+C) trade ~2% per-entry exp noise (which largely
cancels in the softmax) for removing the ACT exp pass; measured
end-to-end rel err ~1.2e-2 vs the 2e-2 gate.

Mask ships once as uint16 [N, ROWS]: fp16 1.0/0.0 rows for A tiles,
fp16 {B, B-30K} for E, fp16 {0, -44320} for C; bitcast on chip.
Mask/xa DMAs are batched and issued from the otherwise-idle GpSimd
sequencer. wt/xt are split into 512-col pieces so the first QK matmul
starts ~1.5us after launch.
"""

import sys

if "/opt/trn_rl_repo" not in sys.path:
    sys.path.insert(0, "/opt/trn_rl_repo")

import numpy as np
import ml_dtypes

import concourse.bass as bass
import concourse.tile as tile
from concourse import bacc, mybir
from concourse.bass_utils import run_bass_kernel_spmd

BF = mybir.dt.bfloat16
F16 = mybir.dt.float16
F32 = mybir.dt.float32
U16 = mybir.dt.uint16
BF_NP = ml_dtypes.bfloat16

H, N, D = 4, 4096, 128
N_CORES = 8
ROWS = N * H // N_CORES          # 2048 rows (n) per core
CHUNK = 1024                     # n columns processed per outer chunk
CHUNKS = ROWS // CHUNK           # 2
M_TILES = N // 128               # 32 tiles along the softmax (m) axis
SUBS = CHUNK // 128              # 8 PV subtiles per chunk
MB = 4                           # m-tiles per batched mask DMA

# Schraudolph constants (fp16 target: 10 mantissa bits)
SK = 1024.0 / float(np.log(2.0))     # 1477.32
BE = 15296.0                          # exp bias for E tiles (fp16-exact)
BC = 15299.0                          # exp bias for C tiles
E_MASKED = -29024.0                   # fp16(BE - 30*SK)
C_MASKED = -44320.0                   # fp16(-30*SK)

# Module-level knobs used by test.py; harmless defaults for grading.
TRACE = False
LAST_EXEC_NS = None

_CACHED_NC = None
_CACHED_BIAS = None


def _classes(has_bias, chunk):
    """Per-m-tile pipeline class: 'A' exact, 'E'/'C' Schraudolph."""
    if has_bias:
        return ["A"] * M_TILES
    cls = ["E"] * M_TILES
    if chunk == 0:
        # First tiles on ACT: compute starts before the mask DMAs land.
        a_set = list(range(0, 4)) + list(range(12, 16)) + list(range(24, 28))
        c_set = (5, 8, 11, 17, 20, 23, 29)
    else:
        # Masks are prefetched by now; lead with DVE-heavy tiles so DVE
        # ramps while ACT finishes the previous chunk.
        a_set = list(range(4, 8)) + list(range(14, 18)) + list(range(24, 28))
        c_set = (0, 2, 9, 11, 19, 29, 31)
    for m in a_set:
        cls[m] = "A"
    for m in c_set:
        cls[m] = "C"
    return cls


def _build_nc(has_bias=False):
    nc = bacc.Bacc("TRN2", target_bir_lowering=False, debug=False,
                   num_devices=N_CORES)
    xt_d = nc.dram_tensor("xt", [128, ROWS], BF, kind="ExternalInput").ap()
    wt_d = nc.dram_tensor("wt", [128, N], BF, kind="ExternalInput").ap()
    xa_d = nc.dram_tensor("xa", [N, D + 1], F16, kind="ExternalInput").ap()
    mk_d = nc.dram_tensor("mk", [N, ROWS], U16, kind="ExternalInput").ap()
    bc_d = nc.dram_tensor("bc", [128, M_TILES], F32, kind="ExternalInput").ap()
    out_d = nc.dram_tensor("out", [ROWS, D], F32, kind="ExternalOutput").ap()

    PRELU = mybir.ActivationFunctionType.Prelu
    EXP = mybir.ActivationFunctionType.Exp
    COPY = mybir.ActivationFunctionType.Copy
    MUL = mybir.AluOpType.mult
    ADD = mybir.AluOpType.add
    MAX = mybir.AluOpType.max

    WPC = 512                        # wt DMA piece columns (4 m-tiles)
    n_wt = N // WPC                  # 8 stationary pieces
    XG = 8                           # m-tiles per xa group
    n_xa = M_TILES // XG             # 4 PV-moving groups

    with tile.TileContext(nc) as tc:
        with (
            tc.tile_pool(name="const", bufs=1) as cpool,
            tc.tile_pool(name="mask", bufs=7) as mpool,
            tc.tile_pool(name="worka", bufs=2) as apool,
            tc.tile_pool(name="worke", bufs=5) as epool,
            tc.tile_pool(name="workc", bufs=3) as qpool,
            tc.tile_pool(name="outp", bufs=3) as opool,
            tc.tile_pool(name="spsum", bufs=2, space="PSUM") as spool,
            tc.tile_pool(name="opsum", bufs=1, space="PSUM") as oppool,
        ):
            wt_sb = [cpool.tile([128, WPC], BF, name=f"wt{i}")
                     for i in range(n_wt)]
            xt_sb = [cpool.tile([128, 512], BF, name=f"xt{i}")
                     for i in range(2 * CHUNKS)]
            bc_sb = cpool.tile([128, M_TILES], F32)
            xa_sb = [cpool.tile([128, XG, D + 1], F16, name=f"xa{i}")
                     for i in range(n_xa)]

            # Warm the ACT LUT set while input DMAs are in flight.
            warm_sb = cpool.tile([1, 1], F32)
            nc.scalar.activation(warm_sb[:], warm_sb[:], EXP)

            # First pieces needed by the first QK matmul go first.
            nc.sync.dma_start(out=wt_sb[0][:], in_=wt_d[:, 0:WPC])
            nc.sync.dma_start(out=xt_sb[0][:], in_=xt_d[:, 0:512])
            if has_bias:
                nc.sync.dma_start(out=bc_sb[:], in_=bc_d[:, :])
            xa_v = xa_d.rearrange("(g j p) d -> g p j d", p=128, j=XG)
            for i in range(1, 2 * CHUNKS):
                nc.sync.dma_start(out=xt_sb[i][:],
                                  in_=xt_d[:, i * 512:(i + 1) * 512])
            for i in range(1, n_wt):
                nc.sync.dma_start(out=wt_sb[i][:],
                                  in_=wt_d[:, i * WPC:(i + 1) * WPC])

            pending_tail = None
            for c in range(CHUNKS):
                CLS = _classes(has_bias, c)
                # consecutive A runs share one Exp instruction
                a_groups, run = [], []
                for m in range(M_TILES):
                    if CLS[m] == "A":
                        run.append(m)
                        if len(run) == 4:
                            a_groups.append(run)
                            run = []
                    elif run:
                        a_groups.append(run)
                        run = []
                if run:
                    a_groups.append(run)
                group_of = {m: g for g in a_groups for m in g}

                o_ps = []

                def o_ap(s, o_ps=o_ps):
                    return o_ps[s // 3][:, (s % 3) * 129:(s % 3) * 129 + 129]

                state = {"first": True}

                def emit_pv(m, lhs_t, c=c, o_ps=o_ps, state=state):
                    nonlocal pending_tail
                    if state["first"]:
                        state["first"] = False
                        if pending_tail is not None:
                            pending_tail()
                            pending_tail = None
                        o_ps.extend(
                            oppool.tile([128, 512], F32, tag=f"oacc{b}",
                                        name=f"oacc{b}_c{c}")
                            for b in range((SUBS + 2) // 3))
                    for s in range(SUBS):
                        nc.tensor.matmul(
                            o_ap(s),
                            lhsT=lhs_t[:, s * 128:(s + 1) * 128],
                            rhs=xa_sb[m // XG][:, m % XG],
                            start=(m == 0 and s % 3 == 0),
                            stop=(m == M_TILES - 1),
                            skip_group_check=True,
                        )

                # Software pipeline: PV matmuls (and the A-tiles' mask
                # multiply) are deferred behind a lag queue so the Tensor
                # queue never sits directly behind a long EW chain.
                pv_q = []   # (ready_step, emit_fn) in m order

                def pop_pv(limit):
                    while pv_q and pv_q[0][0] <= limit:
                        pv_q.pop(0)[1]()

                mkb_sb = None
                lq_sb = None
                mk_aps = {}
                for m in range(M_TILES):
                    # Batched mask DMA, alternating GpSimd / Sync issue.
                    if m % MB == 0:
                        mkb_sb = mpool.tile([128, MB, CHUNK], U16,
                                            tag="mkb", name=f"mkb_c{c}_m{m}")
                        mk_v = mk_d[m * 128:(m + MB) * 128,
                                    c * CHUNK:(c + 1) * CHUNK].rearrange(
                                        "(t p) n -> p t n", p=128)
                        eng = nc.gpsimd if (m // MB) % 2 == 0 else nc.sync
                        eng.dma_start(out=mkb_sb[:], in_=mk_v)
                        if c == 0 and m == 0:
                            nc.gpsimd.dma_start(out=xa_sb[0][:], in_=xa_v[0])
                        if c == 0 and m == MB:
                            for g in range(1, n_xa):
                                nc.gpsimd.dma_start(out=xa_sb[g][:],
                                                    in_=xa_v[g])
                    mk_sb = mkb_sb[:, m % MB]
                    mk_aps[m] = mk_sb

                    # QK: scores^T tile [m=128, n=CHUNK] in PSUM.
                    s_ps = spool.tile([128, CHUNK], F32, tag="s",
                                      name=f"s_c{c}_m{m}")
                    for half in range(CHUNK // 512):
                        nc.tensor.matmul(
                            s_ps[:, half * 512:(half + 1) * 512],
                            lhsT=wt_sb[(m * 128) // WPC]
                                 [:, (m * 128) % WPC:(m * 128) % WPC + 128],
                            rhs=xt_sb[c * 2 + half][:],
                            start=True, stop=True,
                        )

                    cls = CLS[m]
                    if cls == "A":
                        grp = group_of[m]
                        gi = grp.index(m)
                        if gi == 0:
                            lq_sb = apool.tile([128, len(grp), CHUNK], F16,
                                               tag="lq", name=f"lq_c{c}_m{m}")
                        bias = bc_sb[:, m:m + 1] if has_bias else 0.0
                        nc.scalar.activation(lq_sb[:, gi], s_ps[:], PRELU,
                                             bias=bias, scale=1.0, alpha=0.2)
                        if gi == len(grp) - 1:
                            eq_sb = apool.tile([128, len(grp), CHUNK], F16,
                                               tag="eq", name=f"eq_c{c}_m{m}")
                            nc.scalar.activation(eq_sb[:], lq_sb[:], EXP)

                            def mk_a(mj, gj, eq_sb=eq_sb, mk_ap=None):
                                mk_ap = mk_aps[mj]
                                def go():
                                    a_sb = epool.tile(
                                        [128, CHUNK], F16, tag="a",
                                        name=f"a_c{c}_m{mj}")
                                    nc.vector.tensor_mul(
                                        a_sb[:], eq_sb[:, gj],
                                        mk_ap.bitcast(F16))
                                    emit_pv(mj, a_sb[:])
                                return go
                            for gj, mj in enumerate(grp):
                                pv_q.append((m + gj, mk_a(mj, gj)))
                    elif cls == "E":
                        lp_sb = epool.tile([128, CHUNK], F16, tag="lp",
                                           name=f"lp_c{c}_m{m}")
                        nc.scalar.activation(lp_sb[:], s_ps[:], PRELU,
                                             bias=0.0, scale=SK, alpha=0.2)
                        eb_sb = epool.tile([128, CHUNK], U16, tag="eb",
                                           name=f"eb_c{c}_m{m}")
                        nc.vector.tensor_tensor(eb_sb[:], lp_sb[:],
                                                mk_sb.bitcast(F16), ADD)
                        pv_q.append(
                            (m, lambda m=m, eb_sb=eb_sb:
                             emit_pv(m, eb_sb[:].bitcast(F16))))
                    else:  # C
                        t_sb = qpool.tile([128, CHUNK], F16, tag="ct",
                                          name=f"t_c{c}_m{m}")
                        nc.vector.scalar_tensor_tensor(
                            t_sb[:], s_ps[:], 0.2 * SK, mk_sb.bitcast(F16),
                            op0=MUL, op1=ADD)
                        u_sb = qpool.tile([128, CHUNK], F16, tag="cu",
                                          name=f"u_c{c}_m{m}")
                        nc.vector.scalar_tensor_tensor(
                            u_sb[:], t_sb[:], 5.0, t_sb[:],
                            op0=MUL, op1=MAX)
                        cb_sb = qpool.tile([128, CHUNK], U16, tag="cb",
                                           name=f"cb_c{c}_m{m}")
                        nc.vector.tensor_scalar(cb_sb[:], u_sb[:], BC, 0.0,
                                                op0=ADD, op1=MAX)
                        pv_q.append(
                            (m, lambda m=m, cb_sb=cb_sb:
                             emit_pv(m, cb_sb[:].bitcast(F16))))
                    pop_pv(m - 2)
                pop_pv(M_TILES)

                def make_tail(c=c, o_ap=o_ap):
                    def emit_tail():
                        # Division tail: reciprocal of the free row-sum,
                        # scale-copy gathered into one tile per chunk.
                        of_big = opool.tile([128, SUBS, D], F32, tag="ofbig",
                                            name=f"ofbig_c{c}")
                        r_sbs = []
                        for s in range(SUBS):
                            r_sb = opool.tile([128, 1], F32, tag=f"recip{s}",
                                              name=f"recip_c{c}_s{s}")
                            nc.vector.reciprocal(r_sb[:], o_ap(s)[:, 128:129])
                            r_sbs.append(r_sb)
                        for s in range(SUBS):
                            on_act_scale = (c == CHUNKS - 1) and s % 2 == 0
                            if on_act_scale:
                                nc.scalar.activation(of_big[:, s],
                                                     o_ap(s)[:, 0:D],
                                                     COPY, bias=0.0,
                                                     scale=r_sbs[s][:])
                            else:
                                nc.vector.tensor_scalar_mul(of_big[:, s],
                                                            o_ap(s)[:, 0:D],
                                                            r_sbs[s][:])
                        halves = 2 if c == CHUNKS - 1 else 1
                        hs = SUBS // halves
                        for hh in range(halves):
                            row0 = c * CHUNK + hh * hs * 128
                            out_v = out_d[row0:row0 + hs * 128, :].rearrange(
                                "(s p) d -> p s d", p=128)
                            eng = nc.scalar if hh == 1 else nc.sync
                            eng.dma_start(out=out_v,
                                          in_=of_big[:, hh * hs:(hh + 1) * hs])
                    return emit_tail
                pending_tail = make_tail()
            if pending_tail is not None:
                pending_tail()

    nc.compile()
    return nc


def _pack_mask(mask_t_u8, has_bias):
    """mask_t_u8: [N, ROWS] 0/1. Returns uint16-packed per-m-tile rows."""
    out = np.empty(mask_t_u8.shape, np.uint16)
    for c in range(CHUNKS):
        cls = _classes(has_bias, c)
        cols = slice(c * CHUNK, (c + 1) * CHUNK)
        for m in range(M_TILES):
            rows = slice(m * 128, (m + 1) * 128)
            blk = mask_t_u8[rows, cols].astype(np.float32)
            if cls[m] == "A":
                vals = blk
            elif cls[m] == "E":
                vals = np.where(blk > 0, BE, E_MASKED)
            else:
                vals = np.where(blk > 0, 0.0, C_MASKED)
            out[rows, cols] = vals.astype(np.float16).view(np.uint16)
    return out


def kernel(x, W, b, neighbor_mask):
    global _CACHED_NC, _CACHED_BIAS, LAST_EXEC_NS
    x = np.asarray(x, dtype=np.float32)
    W = np.asarray(W, dtype=np.float32)
    b = np.asarray(b, dtype=np.float32)
    mask = np.asarray(neighbor_mask)

    has_bias = bool(np.any(b))
    if _CACHED_NC is None or _CACHED_BIAS != has_bias:
        _CACHED_NC = _build_nc(has_bias=has_bias)
        _CACHED_BIAS = has_bias
    nc = _CACHED_NC

    mask_u8 = mask.astype(np.uint8)
    in_maps = []
    for core in range(N_CORES):
        h, rb = core // 2, core % 2
        r0 = rb * ROWS
        xt = np.ascontiguousarray(x[h, r0:r0 + ROWS].T).astype(BF_NP)
        wt = np.ascontiguousarray(W[h].T).astype(BF_NP)
        xa = np.concatenate(
            [x[h], np.ones((N, 1), np.float32)], axis=1
        ).astype(np.float16)
        mk = _pack_mask(
            np.ascontiguousarray(mask_u8[r0:r0 + ROWS].T), has_bias)
        bc = np.ascontiguousarray(b[h].reshape(M_TILES, 128).T)
        in_maps.append({"xt": xt, "wt": wt, "xa": xa, "mk": mk, "bc": bc})

    res = run_bass_kernel_spmd(nc, in_maps, core_ids=list(range(N_CORES)),
                               trace=TRACE)
    LAST_EXEC_NS = res.exec_time_ns

    out = np.empty((H, N, D), np.float32)
    for core in range(N_CORES):
        h, rb = core // 2, core % 2
        r0 = rb * ROWS
        out[h, r0:r0 + ROWS] = res.results[core]["out"]
    return out


# revision 20
# speedup vs baseline: 1.3231x; 1.0935x over previous
"""Distributed Trainium2 (Bass/Tile) kernel for masked GAT-style attention.

Reference computation (H=4 heads, N=4096 nodes, D=128):
    scores = leaky_relu(x @ W^T + b, 0.2)            # [H, N, N]
    att    = where(mask, softmax(where(mask, scores, -inf)), 0)
    out    = att @ x                                  # [H, N, D]

Sharding: 8 cores = 4 heads x 2 row-blocks of 2048 nodes. Each core
computes out[h, r0:r0+2048] independently (no collectives).

Per-core layout ("transposed scores"): scores^T tiles [m=128 part, n free]
so the PV matmul uses the attention tile directly as the stationary
operand and the softmax row-sum comes for free from an appended
ones-column on x.

The 32 m-tiles per chunk are split into three per-tile pipelines chosen
to balance ACT (1.2 GHz, 1x) and DVE (0.96 GHz, STT=1x, TT=2x @16-bit):
- A (12 tiles, in quads): ACT Prelu from PSUM -> one shared 4-tile Exp
  on ACT -> fp16 mask multiply on DVE (2x TT). Exact exp.
- E (13 tiles): ACT Prelu(scale=K) gives K*leaky(s) in fp16; one DVE
  TT-add with a host-baked fp16 mask tensor {B, B-30K} produces
  Schraudolph exp bits in uint16 (negative results saturate to 0 =
  masked-out entries vanish); bitcast fp16 feeds the PV directly.
- C (7 tiles): all-DVE. STT t=0.2K*s+mask (PSUM read), STT u=max(5t,t),
  then a 4x-mode tensor_scalar add-B -> uint16 Schraudolph bits.
Schraudolph tiles (E+C) trade ~2% per-entry exp noise (which largely
cancels in the softmax) for removing the ACT exp pass; measured
end-to-end rel err ~1.2e-2 vs the 2e-2 gate.

Mask ships once as uint16 [N, ROWS]: fp16 1.0/0.0 rows for A tiles,
fp16 {B, B-30K} for E, fp16 {0, -44320} for C; bitcast on chip.
Mask/xa DMAs are batched and issued from the otherwise-idle GpSimd
sequencer. wt/xt are split into 512-col pieces so the first QK matmul
starts ~1.5us after launch.
"""

import sys

if "/opt/trn_rl_repo" not in sys.path:
    sys.path.insert(0, "/opt/trn_rl_repo")

import numpy as np
import ml_dtypes

import concourse.bass as bass
import concourse.tile as tile
from concourse import bacc, mybir
from concourse.bass_utils import run_bass_kernel_spmd

BF = mybir.dt.bfloat16
F16 = mybir.dt.float16
F32 = mybir.dt.float32
U16 = mybir.dt.uint16
BF_NP = ml_dtypes.bfloat16

H, N, D = 4, 4096, 128
N_CORES = 8
ROWS = N * H // N_CORES          # 2048 rows (n) per core
CHUNK = 1024                     # n columns processed per outer chunk
CHUNKS = ROWS // CHUNK           # 2
M_TILES = N // 128               # 32 tiles along the softmax (m) axis
SUBS = CHUNK // 128              # 8 PV subtiles per chunk
MB = 4                           # m-tiles per batched mask DMA

# Schraudolph constants (fp16 target: 10 mantissa bits)
SK = 1024.0 / float(np.log(2.0))     # 1477.32
BE = 15296.0                          # exp bias for E tiles (fp16-exact)
BC = 15299.0                          # exp bias for C tiles
E_MASKED = -29024.0                   # fp16(BE - 30*SK)
C_MASKED = -44320.0                   # fp16(-30*SK)

# Module-level knobs used by test.py; harmless defaults for grading.
TRACE = False
LAST_EXEC_NS = None

_CACHED_NC = None
_CACHED_BIAS = None


def _classes(has_bias, chunk):
    """Per-m-tile pipeline class: 'A' exact, 'E'/'C' Schraudolph."""
    if has_bias:
        return ["A"] * M_TILES
    cls = ["E"] * M_TILES
    if chunk == 0:
        # First tiles on ACT: compute starts before the mask DMAs land.
        a_set = list(range(0, 8)) + list(range(24, 28))
        c_set = (9, 12, 15, 18, 21, 28, 31)
    else:
        # Masks are prefetched by now; lead with DVE-heavy tiles so DVE
        # ramps while ACT finishes the previous chunk.
        a_set = list(range(4, 8)) + list(range(14, 18)) + list(range(24, 28))
        c_set = (0, 2, 9, 11, 19, 29, 31)
    for m in a_set:
        cls[m] = "A"
    for m in c_set:
        cls[m] = "C"
    return cls


def _build_nc(has_bias=False):
    nc = bacc.Bacc("TRN2", target_bir_lowering=False, debug=False,
                   num_devices=N_CORES)
    xt_d = nc.dram_tensor("xt", [128, ROWS], BF, kind="ExternalInput").ap()
    wt_d = nc.dram_tensor("wt", [128, N], BF, kind="ExternalInput").ap()
    xa_d = nc.dram_tensor("xa", [N, D + 1], F16, kind="ExternalInput").ap()
    mk_d = nc.dram_tensor("mk", [N, ROWS], U16, kind="ExternalInput").ap()
    bc_d = nc.dram_tensor("bc", [128, M_TILES], F32, kind="ExternalInput").ap()
    out_d = nc.dram_tensor("out", [ROWS, D], F32, kind="ExternalOutput").ap()

    PRELU = mybir.ActivationFunctionType.Prelu
    EXP = mybir.ActivationFunctionType.Exp
    COPY = mybir.ActivationFunctionType.Copy
    MUL = mybir.AluOpType.mult
    ADD = mybir.AluOpType.add
    MAX = mybir.AluOpType.max

    WPC = 512                        # wt DMA piece columns (4 m-tiles)
    n_wt = N // WPC                  # 8 stationary pieces
    XG = 8                           # m-tiles per xa group
    n_xa = M_TILES // XG             # 4 PV-moving groups

    with tile.TileContext(nc) as tc:
        with (
            tc.tile_pool(name="const", bufs=1) as cpool,
            tc.tile_pool(name="mask", bufs=5) as mpool,
            tc.tile_pool(name="worka", bufs=2) as apool,
            tc.tile_pool(name="worke", bufs=5) as epool,
            tc.tile_pool(name="workc", bufs=3) as qpool,
            tc.tile_pool(name="outp", bufs=3) as opool,
            tc.tile_pool(name="spsum", bufs=2, space="PSUM") as spool,
            tc.tile_pool(name="opsum", bufs=1, space="PSUM") as oppool,
        ):
            wt_sb = [cpool.tile([128, WPC], BF, name=f"wt{i}")
                     for i in range(n_wt)]
            xt_sb = [cpool.tile([128, 512], BF, name=f"xt{i}")
                     for i in range(2 * CHUNKS)]
            bc_sb = cpool.tile([128, M_TILES], F32)
            xa_sb = [cpool.tile([128, XG, D + 1], F16, name=f"xa{i}")
                     for i in range(n_xa)]

            # Warm the ACT LUT set while input DMAs are in flight.
            warm_sb = cpool.tile([1, 1], F32)
            nc.scalar.activation(warm_sb[:], warm_sb[:], EXP)

            # Only the pieces the first QK matmuls need go up front; the
            # rest are staggered inside the loop so early mask batches
            # are not starved of DMA bandwidth.
            nc.sync.dma_start(out=wt_sb[0][:], in_=wt_d[:, 0:WPC])
            nc.sync.dma_start(out=xt_sb[0][:], in_=xt_d[:, 0:512])
            nc.sync.dma_start(out=xt_sb[1][:], in_=xt_d[:, 512:1024])
            if has_bias:
                nc.sync.dma_start(out=bc_sb[:], in_=bc_d[:, :])
            xa_v = xa_d.rearrange("(g j p) d -> g p j d", p=128, j=XG)

            pending_tail = None
            for c in range(CHUNKS):
                CLS = _classes(has_bias, c)
                # consecutive A runs share one Exp instruction; the first
                # run of chunk 0 is split into pairs so the first PV chain
                # is ready as early as possible.
                a_groups, run = [], []

                def close_run(run, c=c):
                    if not run:
                        return
                    sizes = ([2, 2] if c == 0 and run[0] == 0 else [])
                    i = 0
                    for sz in sizes:
                        if i < len(run):
                            a_groups.append(run[i:i + sz])
                            i += sz
                    while i < len(run):
                        a_groups.append(run[i:i + 4])
                        i += 4

                for m in range(M_TILES):
                    if CLS[m] == "A":
                        run.append(m)
                    elif run:
                        close_run(run)
                        run = []
                if run:
                    close_run(run)
                group_of = {m: g for g in a_groups for m in g}

                o_ps = []

                def o_ap(s, o_ps=o_ps):
                    return o_ps[s // 3][:, (s % 3) * 129:(s % 3) * 129 + 129]

                state = {"first": True}

                def emit_pv(m, lhs_t, c=c, o_ps=o_ps, state=state):
                    nonlocal pending_tail
                    if state["first"]:
                        state["first"] = False
                        if pending_tail is not None:
                            pending_tail()
                            pending_tail = None
                        o_ps.extend(
                            oppool.tile([128, 512], F32, tag=f"oacc{b}",
                                        name=f"oacc{b}_c{c}")
                            for b in range((SUBS + 2) // 3))
                    for s in range(SUBS):
                        nc.tensor.matmul(
                            o_ap(s),
                            lhsT=lhs_t[:, s * 128:(s + 1) * 128],
                            rhs=xa_sb[m // XG][:, m % XG],
                            start=(m == 0 and s % 3 == 0),
                            stop=(m == M_TILES - 1),
                            skip_group_check=True,
                        )

                # Software pipeline: PV matmuls (and the A-tiles' mask
                # multiply) are deferred behind a lag queue so the Tensor
                # queue never sits directly behind a long EW chain. One
                # entry pops per iteration to avoid bursts.
                pv_q = []   # (ready_step, emit_fn) in m order

                def pop_pv(limit, max_n=1):
                    n = 0
                    while pv_q and pv_q[0][0] <= limit and n < max_n:
                        pv_q.pop(0)[1]()
                        n += 1

                # Mask DMA batches: small leading batches on chunk 0 so
                # the first EW consumers aren't starved while the bulk
                # input DMAs stream.
                if c == 0:
                    batches = [(0, 2), (2, 2)] + [(s, 4) for s in
                                                  range(4, M_TILES, 4)]
                else:
                    batches = [(s, 4) for s in range(0, M_TILES, 4)]
                batch_at = {s: (i, sz) for i, (s, sz) in enumerate(batches)}

                lq_sb = None
                mk_aps = {}
                for m in range(M_TILES):
                    pop_pv(m - 1)
                    if m in batch_at:
                        bi, bsz = batch_at[m]
                        mkb_sb = mpool.tile([128, bsz, CHUNK], U16,
                                            tag=f"mkb{bsz}",
                                            name=f"mkb_c{c}_m{m}")
                        mk_v = mk_d[m * 128:(m + bsz) * 128,
                                    c * CHUNK:(c + 1) * CHUNK].rearrange(
                                        "(t p) n -> p t n", p=128)
                        eng = nc.gpsimd if bi % 2 == 0 else nc.sync
                        eng.dma_start(out=mkb_sb[:], in_=mk_v)
                        for mm in range(m, m + bsz):
                            mk_aps[mm] = mkb_sb[:, mm - m]
                    if c == 0:
                        # Staggered prefetch of the remaining inputs.
                        if m in (1, 5, 9, 13, 17, 21, 25):
                            k = (m + 3) // 4
                            nc.sync.dma_start(
                                out=wt_sb[k][:],
                                in_=wt_d[:, k * WPC:(k + 1) * WPC])
                        if m == 1:
                            nc.gpsimd.dma_start(out=xa_sb[0][:], in_=xa_v[0])
                        if m == 8:
                            for g in range(1, n_xa):
                                nc.gpsimd.dma_start(out=xa_sb[g][:],
                                                    in_=xa_v[g])
                        if m in (16, 18):
                            i = 2 + (m - 16) // 2
                            nc.sync.dma_start(
                                out=xt_sb[i][:],
                                in_=xt_d[:, i * 512:(i + 1) * 512])
                    mk_sb = mk_aps[m]

                    # QK: scores^T tile [m=128, n=CHUNK] in PSUM.
                    s_ps = spool.tile([128, CHUNK], F32, tag="s",
                                      name=f"s_c{c}_m{m}")
                    for half in range(CHUNK // 512):
                        nc.tensor.matmul(
                            s_ps[:, half * 512:(half + 1) * 512],
                            lhsT=wt_sb[(m * 128) // WPC]
                                 [:, (m * 128) % WPC:(m * 128) % WPC + 128],
                            rhs=xt_sb[c * 2 + half][:],
                            start=True, stop=True,
                        )

                    cls = CLS[m]
                    if cls == "A":
                        grp = group_of[m]
                        gi = grp.index(m)
                        if gi == 0:
                            lq_sb = apool.tile([128, len(grp), CHUNK], F16,
                                               tag=f"lq{len(grp)}",
                                               name=f"lq_c{c}_m{m}")
                        bias = bc_sb[:, m:m + 1] if has_bias else 0.0
                        nc.scalar.activation(lq_sb[:, gi], s_ps[:], PRELU,
                                             bias=bias, scale=1.0, alpha=0.2)
                        if gi == len(grp) - 1:
                            eq_sb = apool.tile([128, len(grp), CHUNK], F16,
                                               tag=f"eq{len(grp)}",
                                               name=f"eq_c{c}_m{m}")
                            nc.scalar.activation(eq_sb[:], lq_sb[:], EXP)

                            def mk_a(mj, gj, eq_sb=eq_sb, mk_ap=None):
                                mk_ap = mk_aps[mj]
                                def go():
                                    a_sb = epool.tile(
                                        [128, CHUNK], F16, tag="a",
                                        name=f"a_c{c}_m{mj}")
                                    nc.vector.tensor_mul(
                                        a_sb[:], eq_sb[:, gj],
                                        mk_ap.bitcast(F16))
                                    emit_pv(mj, a_sb[:])
                                return go
                            for gj, mj in enumerate(grp):
                                pv_q.append((m + gj, mk_a(mj, gj)))
                    elif cls == "E":
                        lp_sb = epool.tile([128, CHUNK], F16, tag="lp",
                                           name=f"lp_c{c}_m{m}")
                        nc.scalar.activation(lp_sb[:], s_ps[:], PRELU,
                                             bias=0.0, scale=SK, alpha=0.2)
                        eb_sb = epool.tile([128, CHUNK], U16, tag="eb",
                                           name=f"eb_c{c}_m{m}")
                        nc.vector.tensor_tensor(eb_sb[:], lp_sb[:],
                                                mk_sb.bitcast(F16), ADD)
                        pv_q.append(
                            (m, lambda m=m, eb_sb=eb_sb:
                             emit_pv(m, eb_sb[:].bitcast(F16))))
                    else:  # C
                        t_sb = qpool.tile([128, CHUNK], F16, tag="ct",
                                          name=f"t_c{c}_m{m}")
                        nc.vector.scalar_tensor_tensor(
                            t_sb[:], s_ps[:], 0.2 * SK, mk_sb.bitcast(F16),
                            op0=MUL, op1=ADD)
                        u_sb = qpool.tile([128, CHUNK], F16, tag="cu",
                                          name=f"u_c{c}_m{m}")
                        nc.vector.scalar_tensor_tensor(
                            u_sb[:], t_sb[:], 5.0, t_sb[:],
                            op0=MUL, op1=MAX)
                        cb_sb = qpool.tile([128, CHUNK], U16, tag="cb",
                                           name=f"cb_c{c}_m{m}")
                        nc.vector.tensor_scalar(cb_sb[:], u_sb[:], BC, 0.0,
                                                op0=ADD, op1=MAX)
                        pv_q.append(
                            (m, lambda m=m, cb_sb=cb_sb:
                             emit_pv(m, cb_sb[:].bitcast(F16))))
                pop_pv(M_TILES, max_n=len(pv_q))

                def make_tail(c=c, o_ap=o_ap):
                    def emit_tail():
                        # Division tail: reciprocal of the free row-sum,
                        # scale-copy gathered into one tile per chunk.
                        of_big = opool.tile([128, SUBS, D], F32, tag="ofbig",
                                            name=f"ofbig_c{c}")
                        r_sbs = []
                        for s in range(SUBS):
                            r_sb = opool.tile([128, 1], F32, tag=f"recip{s}",
                                              name=f"recip_c{c}_s{s}")
                            nc.vector.reciprocal(r_sb[:], o_ap(s)[:, 128:129])
                            r_sbs.append(r_sb)
                        for s in range(SUBS):
                            on_act_scale = (c == CHUNKS - 1) and s % 2 == 0
                            if on_act_scale:
                                nc.scalar.activation(of_big[:, s],
                                                     o_ap(s)[:, 0:D],
                                                     COPY, bias=0.0,
                                                     scale=r_sbs[s][:])
                            else:
                                nc.vector.tensor_scalar_mul(of_big[:, s],
                                                            o_ap(s)[:, 0:D],
                                                            r_sbs[s][:])
                        halves = 2 if c == CHUNKS - 1 else 1
                        hs = SUBS // halves
                        for hh in range(halves):
                            row0 = c * CHUNK + hh * hs * 128
                            out_v = out_d[row0:row0 + hs * 128, :].rearrange(
                                "(s p) d -> p s d", p=128)
                            eng = nc.scalar if hh == 1 else nc.sync
                            eng.dma_start(out=out_v,
                                          in_=of_big[:, hh * hs:(hh + 1) * hs])
                    return emit_tail
                pending_tail = make_tail()
            if pending_tail is not None:
                pending_tail()

    nc.compile()
    return nc


def _pack_mask(mask_t_u8, has_bias):
    """mask_t_u8: [N, ROWS] 0/1. Returns uint16-packed per-m-tile rows."""
    out = np.empty(mask_t_u8.shape, np.uint16)
    for c in range(CHUNKS):
        cls = _classes(has_bias, c)
        cols = slice(c * CHUNK, (c + 1) * CHUNK)
        for m in range(M_TILES):
            rows = slice(m * 128, (m + 1) * 128)
            blk = mask_t_u8[rows, cols].astype(np.float32)
            if cls[m] == "A":
                vals = blk
            elif cls[m] == "E":
                vals = np.where(blk > 0, BE, E_MASKED)
            else:
                vals = np.where(blk > 0, 0.0, C_MASKED)
            out[rows, cols] = vals.astype(np.float16).view(np.uint16)
    return out


def kernel(x, W, b, neighbor_mask):
    global _CACHED_NC, _CACHED_BIAS, LAST_EXEC_NS
    x = np.asarray(x, dtype=np.float32)
    W = np.asarray(W, dtype=np.float32)
    b = np.asarray(b, dtype=np.float32)
    mask = np.asarray(neighbor_mask)

    has_bias = bool(np.any(b))
    if _CACHED_NC is None or _CACHED_BIAS != has_bias:
        _CACHED_NC = _build_nc(has_bias=has_bias)
        _CACHED_BIAS = has_bias
    nc = _CACHED_NC

    mask_u8 = mask.astype(np.uint8)
    in_maps = []
    for core in range(N_CORES):
        h, rb = core // 2, core % 2
        r0 = rb * ROWS
        xt = np.ascontiguousarray(x[h, r0:r0 + ROWS].T).astype(BF_NP)
        wt = np.ascontiguousarray(W[h].T).astype(BF_NP)
        xa = np.concatenate(
            [x[h], np.ones((N, 1), np.float32)], axis=1
        ).astype(np.float16)
        mk = _pack_mask(
            np.ascontiguousarray(mask_u8[r0:r0 + ROWS].T), has_bias)
        bc = np.ascontiguousarray(b[h].reshape(M_TILES, 128).T)
        in_maps.append({"xt": xt, "wt": wt, "xa": xa, "mk": mk, "bc": bc})

    res = run_bass_kernel_spmd(nc, in_maps, core_ids=list(range(N_CORES)),
                               trace=TRACE)
    LAST_EXEC_NS = res.exec_time_ns

    out = np.empty((H, N, D), np.float32)
    for core in range(N_CORES):
        h, rb = core // 2, core % 2
        r0 = rb * ROWS
        out[h, r0:r0 + ROWS] = res.results[core]["out"]
    return out
